# revision 10
# baseline (speedup 1.0000x reference)
"""Trainium2 Bass kernel for nn_CurvatureLoss (retrieval_knn) — v2.

Host-side: Morton-sorts both clouds (permutation-invariant loss), builds
per-query-tile candidate windows (W1=640 for self-KNN k=10, W3=768 for
cross-KNN k=5) plus bf16 hi/lo-split matmul layouts. The radius mask
(sq-dist 2.5) makes out-of-window neighbors nearly irrelevant; measured
end-to-end rel err is 8.5e-3 on hardware (tolerance 2e-2).

Device-side per 128-query tile: error-compensated K=16 bf16 matmul gives
P' = -sqdist for the window; exact top-10 via max8 + match_replace + max8
(full row); threshold mask (gpsimd AP-scalar is_ge); PE transpose;
index-free neighbor-sum matmul against [hi|1|lo|0] columns. KNN3 builds a
weighted mask U = w*1[P'>=thr5] + alpha0*1[P'>=v1] and gathers curv2 via
the same transpose-matmul machinery against a window table fetched with
gpsimd.ap_gather; the matmul's count column is the normalizer.

Sharding: 8 cores = 2 batches x 4 quarters. curv2 all-gathered across
the 4 cores of each batch. Loss reduced on host from per-core
[2048,3] moved/inter outputs.
"""

import numpy as np
import ml_dtypes

BF16 = ml_dtypes.bfloat16

N = 8192
B = 2
NCORES = 8
QPC = 2048
NQT = 16
W1 = 640           # self-KNN candidate window (multiple of 128)
W3 = 768           # cross-KNN candidate window (multiple of 128)
PADL = (W1 - 128) // 2          # left margin of curv windows
CORE_COLS = QPC + W1 - 128      # padded per-core candidate columns
RADIUS = 2.5
NEG_BIG = -1e30


# ---------------------------------------------------------------- host prep
def _morton(pts, mn, mx, bits=10):
    q = np.clip((pts - mn) / (mx - mn + 1e-9) * (2 ** bits - 1),
                0, 2 ** bits - 1).astype(np.uint64)
    code = np.zeros(len(pts), np.uint64)
    for b in range(bits):
        for c in range(3):
            code |= ((q[:, c] >> np.uint64(b)) & np.uint64(1)) << np.uint64(3 * b + c)
    return code


def _hilo(x):
    hi = x.astype(BF16)
    lo = (x - hi.astype(np.float32)).astype(BF16)
    return hi, lo


def _rt_rows(pts):
    """[16, M] bf16 matmul rhs rows for ref points pts [M,3]."""
    M = len(pts)
    r2 = np.sum(pts.astype(np.float64) ** 2, 1).astype(np.float32)
    chi, clo = _hilo(pts)
    r2h, r2l = _hilo(r2)
    rt = np.zeros((16, M), BF16)
    rt[0:3] = chi.T
    rt[3:6] = clo.T
    rt[6:9] = chi.T
    rt[9:12] = clo.T
    rt[12] = (-0.5 * r2h.astype(np.float32)).astype(BF16)
    rt[13] = (-0.5 * r2l.astype(np.float32)).astype(BF16)
    return rt


def _qt_rows(pts):
    """[16, M] bf16 matmul lhsT rows for query points pts [M,3]."""
    M = len(pts)
    qhi, qlo = _hilo(pts)
    qt = np.zeros((16, M), BF16)
    qt[0:3] = qhi.T
    qt[3:6] = qhi.T
    qt[6:9] = qlo.T
    qt[9:12] = qlo.T
    qt[12] = 1.0
    qt[13] = 1.0
    return qt


def _hl_cols(pts):
    """[M, 8] bf16 gather columns [hi(3), 1, lo(3), 0]."""
    hi, lo = _hilo(pts)
    hl = np.zeros((len(pts), 8), BF16)
    hl[:, 0:3] = hi
    hl[:, 3] = 1.0
    hl[:, 4:7] = lo
    return hl


def _q2_wide(pts):
    q2 = np.sum(pts.astype(np.float64) ** 2, 1).astype(np.float32)
    return np.ascontiguousarray(q2.reshape(NQT, 128).T)


def _win_tiles(sorted_pts, offs, W):
    """rt [T,16,W] + hl source rows for per-tile windows at offs."""
    rt = np.stack([_rt_rows(sorted_pts[o:o + W]) for o in offs])
    return np.ascontiguousarray(rt)


def prepare_inputs(pc_source, pc_target, pred_flow):
    """Returns (in_maps, metas). metas[c] = o3 permutation for loss pairing."""
    warp = pc_source + pred_flow
    in_maps = [None] * NCORES
    metas = [None] * NCORES
    for b in range(B):
        t, s, w = pc_target[b], pc_source[b], warp[b]
        mn = np.minimum(t.min(0), w.min(0))
        mx = np.maximum(t.max(0), w.max(0))
        ot = np.argsort(_morton(t, mn, mx), kind="stable")
        ts = t[ot]
        tcodes = _morton(ts, mn, mx)
        osrc = np.argsort(_morton(s, s.min(0), s.max(0)), kind="stable")
        ss = s[osrc]
        ws = w[osrc]
        dummy = np.full((PADL, 3), 1e3, np.float32)
        ts_pad = np.concatenate([dummy, ts, dummy])
        ss_pad = np.concatenate([dummy, ss, dummy])
        ws_pad = np.concatenate([dummy, ws, dummy])
        hl_t_pad = _hl_cols(ts_pad)
        hl_w_pad = _hl_cols(ws_pad)
        rt_t_pad = _rt_rows(ts_pad)
        rt_s_pad = _rt_rows(ss_pad)
        rtt_full = _rt_rows(ts)
        # offs3 windows slice rtt_full (never clipped below 0 / above N-W3)
        for qi in range(4):
            c = 4 * b + qi
            g0 = QPC * qi
            # curv windows: padded array cols [g0, g0 + CORE_COLS)
            tq = ts[g0:g0 + QPC]
            sq = ss[g0:g0 + QPC]
            wq = ws[g0:g0 + QPC]
            # KNN3: warped queries (local morton order), target refs
            o3 = np.argsort(_morton(wq, mn, mx), kind="stable")
            wq3 = wq[o3]
            pos = np.searchsorted(tcodes, _morton(wq3, mn, mx))
            offs3 = [int(np.clip(int(np.median(pos[128 * ti:128 * ti + 128]))
                                 - W3 // 2, 0, N - W3)) // 128 * 128
                     for ti in range(NQT)]
            i3 = np.zeros((128, NQT), np.int16)
            for ti, o in enumerate(offs3):
                base = o // 128
                for j in range(16):
                    i3[j::16, ti] = base + min(j, W3 // 128 - 1)
            in_maps[c] = {
                "rtc1": np.ascontiguousarray(rt_t_pad[:, g0:g0 + CORE_COLS]),
                "rtc2": np.ascontiguousarray(rt_s_pad[:, g0:g0 + CORE_COLS]),
                "rt3": np.ascontiguousarray(
                    np.stack([rtt_full[:, o:o + W3] for o in offs3])),
                "hlc1": np.ascontiguousarray(hl_t_pad[g0:g0 + CORE_COLS]),
                "hlc2": np.ascontiguousarray(hl_w_pad[g0:g0 + CORE_COLS]),
                "qt1": np.ascontiguousarray(_qt_rows(tq)),
                "qt2": np.ascontiguousarray(_qt_rows(sq)),
                "qt3": np.ascontiguousarray(_qt_rows(wq3)),
                "q2_1": _q2_wide(tq),
                "q2_2": _q2_wide(sq),
                "q2_3": _q2_wide(wq3),
                "ctr1": np.ascontiguousarray(tq),
                "ctr2": np.ascontiguousarray(wq),
                "i3": np.ascontiguousarray(i3),
            }
            metas[c] = o3
    return in_maps, metas


# ---------------------------------------------------------------- device
def _build(debug=0):
    import concourse.bacc as bacc
    import concourse.mybir as mybir
    import concourse.tile as tile
    from concourse.masks import make_identity

    f32 = mybir.dt.float32
    bf16 = mybir.dt.bfloat16
    u32 = mybir.dt.uint32
    Alu = mybir.AluOpType
    Act = mybir.ActivationFunctionType
    X = mybir.AxisListType.X

    C1 = W1 // 128     # transpose chunks per curv tile
    C3 = W3 // 128
    CH = N // 128

    nc = bacc.Bacc(None, num_devices=NCORES)

    CORE_COLS = QPC + W1 - 128
    rtc1 = nc.dram_tensor("rtc1", [16, CORE_COLS], bf16, kind="ExternalInput")
    rtc2 = nc.dram_tensor("rtc2", [16, CORE_COLS], bf16, kind="ExternalInput")
    rt3 = nc.dram_tensor("rt3", [NQT, 16, W3], bf16, kind="ExternalInput")
    hlc1 = nc.dram_tensor("hlc1", [CORE_COLS, 8], bf16, kind="ExternalInput")
    hlc2 = nc.dram_tensor("hlc2", [CORE_COLS, 8], bf16, kind="ExternalInput")
    qt1 = nc.dram_tensor("qt1", [16, QPC], bf16, kind="ExternalInput")
    qt2 = nc.dram_tensor("qt2", [16, QPC], bf16, kind="ExternalInput")
    qt3 = nc.dram_tensor("qt3", [16, QPC], bf16, kind="ExternalInput")
    q2_1 = nc.dram_tensor("q2_1", [128, NQT], f32, kind="ExternalInput")
    q2_2 = nc.dram_tensor("q2_2", [128, NQT], f32, kind="ExternalInput")
    q2_3 = nc.dram_tensor("q2_3", [128, NQT], f32, kind="ExternalInput")
    ctr1 = nc.dram_tensor("ctr1", [QPC, 3], f32, kind="ExternalInput")
    ctr2 = nc.dram_tensor("ctr2", [QPC, 3], f32, kind="ExternalInput")
    i3_d = nc.dram_tensor("i3", [128, NQT], mybir.dt.int16,
                          kind="ExternalInput")
    moved_out = nc.dram_tensor("moved_part", [QPC, 3], f32,
                               kind="ExternalOutput")
    inter_out = nc.dram_tensor("inter_part", [QPC, 3], f32,
                               kind="ExternalOutput")
    dbg = {}
    if debug:
        dbg["curv2_part"] = nc.dram_tensor("curv2_part", [QPC, 3], f32,
                                           kind="ExternalOutput")
        dbg["v16_dbg"] = nc.dram_tensor("v16_dbg", [QPC, 16], f32,
                                        kind="ExternalOutput")

    with tile.TileContext(nc) as tc:
        with (
            tc.tile_pool(name="persist", bufs=1) as pers,
            tc.tile_pool(name="dram", bufs=1, space="DRAM") as dram,
            tc.tile_pool(name="pRT", bufs=4) as pRT,
            tc.tile_pool(name="pHL", bufs=3) as pHL,
            tc.tile_pool(name="pP", bufs=4) as pP,
            tc.tile_pool(name="pW", bufs=4) as pW,
            tc.tile_pool(name="pWT", bufs=4) as pWT,
            tc.tile_pool(name="small", bufs=8) as small,
            tc.tile_pool(name="psD", bufs=2, space="PSUM") as psD,
            tc.tile_pool(name="psT", bufs=2, space="PSUM") as psT,
            tc.tile_pool(name="psS", bufs=2, space="PSUM") as psS,
            tc.tile_pool(name="psK3", bufs=1, space="PSUM") as psK3,
        ):
            # ---------------- constants ----------------
            ident_bf = pers.tile([128, 128], bf16)
            make_identity(nc, ident_bf)
            ident_f32 = pers.tile([128, 128], f32)
            make_identity(nc, ident_f32)
            ones128 = pers.tile([128, 1], f32)
            nc.vector.memset(ones128[:], 1.0)

            # ---------------- persistent loads ----------------
            def load_qt(src, tag):
                t = pers.tile([16, QPC], bf16, tag=tag)
                nc.sync.dma_start(t[:], src[:])
                return t

            def load_rtc(src, tag):
                t = pers.tile([16, CORE_COLS], bf16, tag=tag)
                nc.sync.dma_start(t[:], src[:])
                return t

            QT1 = load_qt(qt1, "QT1")
            QT2 = load_qt(qt2, "QT2")
            QT3 = load_qt(qt3, "QT3")
            RTC = {1: load_rtc(rtc1, "RTC1"), 2: load_rtc(rtc2, "RTC2")}

            def load_hlc(src, tag):
                t = pers.tile([128, CORE_COLS // 128, 8], bf16, tag=tag)
                nc.sync.dma_start(
                    t[:], src.rearrange("(ci p) c -> p ci c", p=128))
                return t

            HLC = {1: load_hlc(hlc1, "HLC1"), 2: load_hlc(hlc2, "HLC2")}

            def load_128(src, ncol, dt, tag):
                t = pers.tile([128, ncol], dt, tag=tag)
                nc.sync.dma_start(t[:], src[:])
                return t

            Q2 = {1: load_128(q2_1, NQT, f32, "q21"),
                  2: load_128(q2_2, NQT, f32, "q22"),
                  3: load_128(q2_3, NQT, f32, "q23")}
            I3 = load_128(i3_d, NQT, mybir.dt.int16, "i3")

            def load_wide3(src, tag):
                t = pers.tile([128, NQT, 3], f32, tag=tag)
                nc.sync.dma_start(t[:], src.rearrange("(t p) c -> p t c", p=128))
                return t

            CTR1 = load_wide3(ctr1, "ctr1")
            CTR2 = load_wide3(ctr2, "ctr2")

            # negated q2 (evac bias) per stage: [128, NQT]
            def neg_of(t, tag):
                o = pers.tile([128, NQT], f32, tag=tag)
                nc.vector.tensor_scalar_mul(o[:], t[:], -1.0)
                return o

            NQ2 = {k: neg_of(v, f"nq2{k}") for k, v in Q2.items()}

            ag_in = dram.tile([128, NQT, 3], f32)
            ag_out = dram.tile([512, NQT, 3], f32)

            # ---------------- curv stage tile ----------------
            def emit_curv(t, qt_sb, stage, out_cb):
                W = W1
                CW = C1
                RT = RTC[stage]
                HL = HLC[stage]

                pd = psD.tile([128, W], f32, tag="pd")
                for g0 in range(0, W, 512):
                    gw = min(512, W - g0)
                    nc.tensor.matmul(
                        pd[:, g0:g0 + gw],
                        qt_sb[:, t * 128:(t + 1) * 128],
                        RT[:, t * 128 + g0:t * 128 + g0 + gw],
                        start=True, stop=True)
                P = pP.tile([128, W], bf16, tag="P")
                nc.scalar.activation(P[:], pd[:], Act.Identity, scale=2.0,
                                     bias=NQ2[stage][:, t:t + 1])
                v8 = small.tile([128, 8], f32, tag="v8")
                nc.vector.max(v8[:], P[:])
                P2 = pP.tile([128, W], bf16, tag="P2")
                nc.vector.match_replace(P2[:], v8[:], P[:], NEG_BIG)
                v16 = small.tile([128, 8], f32, tag="v16")
                nc.vector.max(v16[:], P2[:])
                thr = small.tile([128, 1], f32, tag="thr")
                nc.vector.tensor_scalar_max(thr[:], v16[:, 1:2], -RADIUS)
                Wm = pW.tile([128, W], bf16, tag="W")
                nc.gpsimd.tensor_scalar(Wm[:], P[:], thr[:], None, Alu.is_ge)
                ps = psS.tile([128, 4], f32, tag="psumS")
                pt = psT.tile([128, CW, 128], bf16, tag="psumT")
                for j in range(CW):
                    nc.tensor.transpose(pt[:, j, :],
                                        Wm[:, j * 128:(j + 1) * 128],
                                        ident_bf[:])
                WT = pWT.tile([128, CW, 128], bf16, tag="WT")
                nc.scalar.activation(WT[:], pt[:], Act.Copy)
                # hi and lo halves accumulate into the same 4 psum columns
                for j in range(CW):
                    for h in range(2):
                        nc.tensor.matmul(
                            ps[:], WT[:, j, :],
                            HL[:, t + j, h * 4:(h + 1) * 4],
                            start=(j == 0 and h == 0),
                            stop=(j == CW - 1 and h == 1))
                S4 = small.tile([128, 4], f32, tag="S4")
                nc.scalar.activation(S4[:], ps[:], Act.Copy, scale=-1.0 / 9.0)
                ctr = CTR1 if stage == 1 else CTR2
                # S4 = -[sum,count]/9, so ctr*S4[3]-S4[0:3] = (sum-count*ctr)/9
                curv = small.tile([128, 3], f32, tag="curv")
                nc.vector.scalar_tensor_tensor(
                    curv[:], ctr[:, t, :], S4[:, 3:4], S4[:, 0:3],
                    Alu.mult, Alu.subtract)
                out_cb(curv, v8, v16)

            # ---------------- KNN1: curvature of target ----------------
            for t in range(NQT):
                def cb1(curv, v8, v16, t=t):
                    nc.sync.dma_start(ag_in[:, t, :], curv[:])
                    if debug:
                        nc.sync.dma_start(
                            dbg["curv2_part"][t * 128:(t + 1) * 128, :],
                            curv[:])
                        nc.sync.dma_start(
                            dbg["v16_dbg"][t * 128:(t + 1) * 128, 0:8], v8[:])
                        nc.sync.dma_start(
                            dbg["v16_dbg"][t * 128:(t + 1) * 128, 8:16],
                            v16[:])
                emit_curv(t, QT1, 1, cb1)

            # ---------------- KNN2: curvature of source/warped ----------
            # (AllGather issued after tile 4 so it overlaps KNN2/KNN3 compute
            #  without stalling the gpsimd queue at a stage boundary.)
            for t in range(NQT):
                def cb2(curv, v8, v16, t=t):
                    nc.sync.dma_start(moved_out[t * 128:(t + 1) * 128, :],
                                      curv[:])
                emit_curv(t, QT2, 2, cb2)
                if t == 0:
                    nc.gpsimd.collective_compute(
                        "AllGather", mybir.AluOpType.bypass,
                        replica_groups=[[0, 1, 2, 3], [4, 5, 6, 7]],
                        ins=[ag_in.opt()], outs=[ag_out.opt()])


            # curv2 gather table [l, h, 4] = [curv2_bf16(3), 1]; bf16-only is
            # fine here (normalized weighted average, no cancellation) and
            # building it gpsimd-only avoids DVE head-of-line blocking while
            # the AllGather completes.
            Ctab_f = pers.tile([128, CH, 3], f32)
            nc.sync.dma_start(
                Ctab_f[:].rearrange("p (k t) c -> p k t c", k=4),
                ag_out[:].rearrange("(k p) t c -> p k t c", p=128))
            CtabHL = pers.tile([128, CH, 4], bf16)
            nc.vector.tensor_copy(CtabHL[:, :, 0:3], Ctab_f[:])
            nc.vector.memset(CtabHL[:, :, 3:4], 1.0)

            # ---------------- KNN3: interp k=5 ----------------
            for t in range(NQT):
                RT = pRT.tile([16, W3], bf16, tag="rt3")
                nc.sync.dma_start(RT[:], rt3[t])
                RTf = RT[:]
                P = pP.tile([128, W3], bf16, tag="P3")
                dd = pP.tile([128, W3], bf16, tag="dd")
                for h0 in range(0, W3, 1024):
                    hw = min(1024, W3 - h0)
                    pd = psD.tile([128, 1024], f32, tag="pd")
                    for g0 in range(0, hw, 512):
                        gw = min(512, hw - g0)
                        nc.tensor.matmul(
                            pd[:, g0:g0 + gw],
                            QT3[:, t * 128:(t + 1) * 128],
                            RTf[:, h0 + g0:h0 + g0 + gw],
                            start=True, stop=True)
                    nc.scalar.activation(
                        P[:, h0:h0 + hw], pd[:, 0:hw],
                        Act.Identity, scale=2.0, bias=NQ2[3][:, t:t + 1])
                    # dd = +sqdist directly from the same PSUM (ACT, not DVE)
                    nc.scalar.activation(
                        dd[:, h0:h0 + hw], pd[:, 0:hw],
                        Act.Identity, scale=-2.0, bias=Q2[3][:, t:t + 1])
                v8 = small.tile([128, 8], f32, tag="v8")
                nc.vector.max(v8[:], P[:])
                # window hl gather table [128, 16, 4] via ap_gather
                hlwin = pW.tile([128, 16, 4], bf16, tag="hlwin")
                nc.gpsimd.ap_gather(hlwin[:], CtabHL[:], I3[:, t:t + 1],
                                    128, CH, 4, 16)
                # weights from v8: alpha0' = sum of out-of-radius top-5 weights
                d8 = small.tile([128, 8], f32, tag="d8")
                nc.vector.tensor_scalar(d8[:], v8[:], -1.0, 1e-8, Alu.mult,
                                        Alu.add)
                w8 = small.tile([128, 5], f32, tag="w8")
                nc.vector.reciprocal(w8[:], d8[:, 0:5])
                w_out = small.tile([128, 5], f32, tag="w_out")
                nc.vector.scalar_tensor_tensor(w_out[:], d8[:, 0:5],
                                               RADIUS + 1e-8, w8[:],
                                               Alu.is_gt, Alu.mult)
                alpha0 = small.tile([128, 1], f32, tag="alpha0")
                nc.vector.tensor_reduce(alpha0[:], w_out[:], X, Alu.add)
                thrm = small.tile([128, 1], f32, tag="thrm")
                nc.vector.tensor_scalar_max(thrm[:], v8[:, 4:5], -RADIUS)
                # w_all = 1/dd (dd==0 impossible: cross-cloud, bf16 evac)
                w_all = pP.tile([128, W3], bf16, tag="w_all")
                with nc.allow_low_precision(reason="knn3 weights, 0.4% ok"):
                    nc.vector.reciprocal(w_all[:], dd[:])
                # U = w_all*1[P'>=thrm] + alpha0*1[P'>=v1]
                msk = pW.tile([128, W3], bf16, tag="msk")
                nc.gpsimd.tensor_scalar(msk[:], P[:], thrm[:], None, Alu.is_ge)
                U = pW.tile([128, W3], bf16, tag="U")
                nc.vector.tensor_mul(U[:], msk[:], w_all[:])
                M1 = pW.tile([128, W3], bf16, tag="M1")
                nc.vector.tensor_scalar(M1[:], P[:], v8[:, 0:1], alpha0[:],
                                        Alu.is_ge, Alu.mult)
                # transpose U and M1 chunks; both accumulate into one psum
                ps = psS.tile([128, 4], f32, tag="psumS")
                TCH = 2 * C3
                for half in range((TCH + 7) // 8):
                    nch = min(8, TCH - 8 * half)
                    pt = psT.tile([128, 8, 128], bf16, tag="psumT")
                    for j in range(nch):
                        c = half * 8 + j
                        src = U if c < C3 else M1
                        cc = c % C3
                        nc.tensor.transpose(
                            pt[:, j, :], src[:, cc * 128:(cc + 1) * 128],
                            ident_bf[:])
                    UT = pWT.tile([128, 8, 128], bf16, tag="WT")
                    nc.scalar.activation(UT[:, 0:nch, :], pt[:, 0:nch, :],
                                         Act.Copy)
                    for j in range(nch):
                        c = half * 8 + j
                        nc.tensor.matmul(ps[:], UT[:, j, :],
                                         hlwin[:, c % C3, :],
                                         start=(c == 0), stop=(c == TCH - 1))
                S4 = small.tile([128, 4], f32, tag="S4k3")
                nc.scalar.activation(S4[:], ps[:], Act.Copy)
                winv = small.tile([128, 1], f32, tag="winv")
                nc.vector.reciprocal(winv[:], S4[:, 3:4])
                inter = small.tile([128, 3], f32, tag="inter")
                nc.vector.tensor_scalar(inter[:], S4[:, 0:3], winv[:], None,
                                        Alu.mult)
                nc.sync.dma_start(inter_out[t * 128:(t + 1) * 128, :],
                                  inter[:])

    nc.compile()
    return nc


_CACHED = {}


def _get_program(debug=0):
    if debug not in _CACHED:
        _CACHED[debug] = _build(debug)
    return _CACHED[debug]


def kernel(pc_source, pc_target, pred_flow):
    from concourse.bass_utils import run_bass_kernel_spmd

    pc_source = np.asarray(pc_source, dtype=np.float32)
    pc_target = np.asarray(pc_target, dtype=np.float32)
    pred_flow = np.asarray(pred_flow, dtype=np.float32)
    nc = _get_program()
    in_maps, metas = prepare_inputs(pc_source, pc_target, pred_flow)
    res = run_bass_kernel_spmd(nc, in_maps, core_ids=list(range(NCORES)))
    total = 0.0
    for c in range(NCORES):
        moved = res.results[c]["moved_part"]
        inter = res.results[c]["inter_part"]
        diff = inter.astype(np.float64) - moved[metas[c]].astype(np.float64)
        total += float((diff ** 2).sum())
    return np.asarray(np.float32(total / B))


# revision 11
# speedup vs baseline: 1.0328x; 1.0328x over previous
"""Trainium2 Bass kernel for nn_CurvatureLoss (retrieval_knn) — v2.

Host-side: Morton-sorts both clouds (permutation-invariant loss), builds
per-query-tile candidate windows (W1=640 for self-KNN k=10, W3=768 for
cross-KNN k=5) plus bf16 hi/lo-split matmul layouts. The radius mask
(sq-dist 2.5) makes out-of-window neighbors nearly irrelevant; measured
end-to-end rel err is 8.5e-3 on hardware (tolerance 2e-2).

Device-side per 128-query tile: error-compensated K=16 bf16 matmul gives
P' = -sqdist for the window; exact top-10 via max8 + match_replace + max8
(full row); threshold mask (gpsimd AP-scalar is_ge); PE transpose;
index-free neighbor-sum matmul against [hi|1|lo|0] columns. KNN3 builds a
weighted mask U = w*1[P'>=thr5] + alpha0*1[P'>=v1] and gathers curv2 via
the same transpose-matmul machinery against a window table fetched with
gpsimd.ap_gather; the matmul's count column is the normalizer.

Sharding: 8 cores = 2 batches x 4 quarters. curv2 all-gathered across
the 4 cores of each batch. Loss reduced on host from per-core
[2048,3] moved/inter outputs.
"""

import numpy as np
import ml_dtypes

BF16 = ml_dtypes.bfloat16

N = 8192
B = 2
NCORES = 8
QPC = 2048
NQT = 16
W1 = 640           # self-KNN candidate window (multiple of 128)
W3 = 768           # cross-KNN candidate window (multiple of 128)
PADL = (W1 - 128) // 2          # left margin of curv windows
CORE_COLS = QPC + W1 - 128      # padded per-core candidate columns
RADIUS = 2.5
NEG_BIG = -1e30


# ---------------------------------------------------------------- host prep
def _morton(pts, mn, mx, bits=10):
    q = np.clip((pts - mn) / (mx - mn + 1e-9) * (2 ** bits - 1),
                0, 2 ** bits - 1).astype(np.uint64)
    code = np.zeros(len(pts), np.uint64)
    for b in range(bits):
        for c in range(3):
            code |= ((q[:, c] >> np.uint64(b)) & np.uint64(1)) << np.uint64(3 * b + c)
    return code


def _hilo(x):
    hi = x.astype(BF16)
    lo = (x - hi.astype(np.float32)).astype(BF16)
    return hi, lo


def _rt_rows(pts):
    """[16, M] bf16 matmul rhs rows for ref points pts [M,3]."""
    M = len(pts)
    r2 = np.sum(pts.astype(np.float64) ** 2, 1).astype(np.float32)
    chi, clo = _hilo(pts)
    r2h, r2l = _hilo(r2)
    rt = np.zeros((16, M), BF16)
    rt[0:3] = chi.T
    rt[3:6] = clo.T
    rt[6:9] = chi.T
    rt[9:12] = clo.T
    rt[12] = (-0.5 * r2h.astype(np.float32)).astype(BF16)
    rt[13] = (-0.5 * r2l.astype(np.float32)).astype(BF16)
    return rt


def _qt_rows(pts):
    """[16, M] bf16 matmul lhsT rows for query points pts [M,3]."""
    M = len(pts)
    qhi, qlo = _hilo(pts)
    qt = np.zeros((16, M), BF16)
    qt[0:3] = qhi.T
    qt[3:6] = qhi.T
    qt[6:9] = qlo.T
    qt[9:12] = qlo.T
    qt[12] = 1.0
    qt[13] = 1.0
    return qt


def _hl_cols(pts):
    """[M, 8] bf16 gather columns [hi(3), 1, lo(3), 0]."""
    hi, lo = _hilo(pts)
    hl = np.zeros((len(pts), 8), BF16)
    hl[:, 0:3] = hi
    hl[:, 3] = 1.0
    hl[:, 4:7] = lo
    return hl


def _q2_wide(pts):
    q2 = np.sum(pts.astype(np.float64) ** 2, 1).astype(np.float32)
    return np.ascontiguousarray(q2.reshape(NQT, 128).T)


def _win_tiles(sorted_pts, offs, W):
    """rt [T,16,W] + hl source rows for per-tile windows at offs."""
    rt = np.stack([_rt_rows(sorted_pts[o:o + W]) for o in offs])
    return np.ascontiguousarray(rt)


def prepare_inputs(pc_source, pc_target, pred_flow):
    """Returns (in_maps, metas). metas[c] = o3 permutation for loss pairing."""
    warp = pc_source + pred_flow
    in_maps = [None] * NCORES
    metas = [None] * NCORES
    for b in range(B):
        t, s, w = pc_target[b], pc_source[b], warp[b]
        mn = np.minimum(t.min(0), w.min(0))
        mx = np.maximum(t.max(0), w.max(0))
        ot = np.argsort(_morton(t, mn, mx), kind="stable")
        ts = t[ot]
        tcodes = _morton(ts, mn, mx)
        osrc = np.argsort(_morton(s, s.min(0), s.max(0)), kind="stable")
        ss = s[osrc]
        ws = w[osrc]
        dummy = np.full((PADL, 3), 1e3, np.float32)
        ts_pad = np.concatenate([dummy, ts, dummy])
        ss_pad = np.concatenate([dummy, ss, dummy])
        ws_pad = np.concatenate([dummy, ws, dummy])
        hl_t_pad = _hl_cols(ts_pad)
        hl_w_pad = _hl_cols(ws_pad)
        rt_t_pad = _rt_rows(ts_pad)
        rt_s_pad = _rt_rows(ss_pad)
        rtt_full = _rt_rows(ts)
        # offs3 windows slice rtt_full (never clipped below 0 / above N-W3)
        for qi in range(4):
            c = 4 * b + qi
            g0 = QPC * qi
            # curv windows: padded array cols [g0, g0 + CORE_COLS)
            tq = ts[g0:g0 + QPC]
            sq = ss[g0:g0 + QPC]
            wq = ws[g0:g0 + QPC]
            # KNN3: warped queries (local morton order), target refs
            o3 = np.argsort(_morton(wq, mn, mx), kind="stable")
            wq3 = wq[o3]
            pos = np.searchsorted(tcodes, _morton(wq3, mn, mx))
            offs3 = [int(np.clip(int(np.median(pos[128 * ti:128 * ti + 128]))
                                 - W3 // 2, 0, N - W3)) // 128 * 128
                     for ti in range(NQT)]
            i3 = np.zeros((128, NQT), np.int16)
            for ti, o in enumerate(offs3):
                base = o // 128
                for j in range(16):
                    i3[j::16, ti] = base + min(j, W3 // 128 - 1)
            in_maps[c] = {
                "rtc1": np.ascontiguousarray(rt_t_pad[:, g0:g0 + CORE_COLS]),
                "rtc2": np.ascontiguousarray(rt_s_pad[:, g0:g0 + CORE_COLS]),
                "rt3": np.ascontiguousarray(
                    np.stack([rtt_full[:, o:o + W3] for o in offs3])),
                "hlc1": np.ascontiguousarray(hl_t_pad[g0:g0 + CORE_COLS]),
                "hlc2": np.ascontiguousarray(hl_w_pad[g0:g0 + CORE_COLS]),
                "qt1": np.ascontiguousarray(_qt_rows(tq)),
                "qt2": np.ascontiguousarray(_qt_rows(sq)),
                "qt3": np.ascontiguousarray(_qt_rows(wq3)),
                "q2_1": _q2_wide(tq),
                "q2_2": _q2_wide(sq),
                "q2_3": _q2_wide(wq3),
                "ctr1": np.ascontiguousarray(tq),
                "ctr2": np.ascontiguousarray(wq),
                "i3": np.ascontiguousarray(i3),
            }
            metas[c] = o3
    return in_maps, metas


# ---------------------------------------------------------------- device
def _build(debug=0):
    import concourse.bacc as bacc
    import concourse.mybir as mybir
    import concourse.tile as tile
    from concourse.masks import make_identity

    f32 = mybir.dt.float32
    bf16 = mybir.dt.bfloat16
    u32 = mybir.dt.uint32
    Alu = mybir.AluOpType
    Act = mybir.ActivationFunctionType
    X = mybir.AxisListType.X

    C1 = W1 // 128     # transpose chunks per curv tile
    C3 = W3 // 128
    CH = N // 128

    nc = bacc.Bacc(None, num_devices=NCORES)

    CORE_COLS = QPC + W1 - 128
    rtc1 = nc.dram_tensor("rtc1", [16, CORE_COLS], bf16, kind="ExternalInput")
    rtc2 = nc.dram_tensor("rtc2", [16, CORE_COLS], bf16, kind="ExternalInput")
    rt3 = nc.dram_tensor("rt3", [NQT, 16, W3], bf16, kind="ExternalInput")
    hlc1 = nc.dram_tensor("hlc1", [CORE_COLS, 8], bf16, kind="ExternalInput")
    hlc2 = nc.dram_tensor("hlc2", [CORE_COLS, 8], bf16, kind="ExternalInput")
    qt1 = nc.dram_tensor("qt1", [16, QPC], bf16, kind="ExternalInput")
    qt2 = nc.dram_tensor("qt2", [16, QPC], bf16, kind="ExternalInput")
    qt3 = nc.dram_tensor("qt3", [16, QPC], bf16, kind="ExternalInput")
    q2_1 = nc.dram_tensor("q2_1", [128, NQT], f32, kind="ExternalInput")
    q2_2 = nc.dram_tensor("q2_2", [128, NQT], f32, kind="ExternalInput")
    q2_3 = nc.dram_tensor("q2_3", [128, NQT], f32, kind="ExternalInput")
    ctr1 = nc.dram_tensor("ctr1", [QPC, 3], f32, kind="ExternalInput")
    ctr2 = nc.dram_tensor("ctr2", [QPC, 3], f32, kind="ExternalInput")
    i3_d = nc.dram_tensor("i3", [128, NQT], mybir.dt.int16,
                          kind="ExternalInput")
    moved_out = nc.dram_tensor("moved_part", [QPC, 3], f32,
                               kind="ExternalOutput")
    inter_out = nc.dram_tensor("inter_part", [QPC, 3], f32,
                               kind="ExternalOutput")
    dbg = {}
    if debug:
        dbg["curv2_part"] = nc.dram_tensor("curv2_part", [QPC, 3], f32,
                                           kind="ExternalOutput")
        dbg["v16_dbg"] = nc.dram_tensor("v16_dbg", [QPC, 16], f32,
                                        kind="ExternalOutput")

    with tile.TileContext(nc) as tc:
        with (
            tc.tile_pool(name="persist", bufs=1) as pers,
            tc.tile_pool(name="dram", bufs=1, space="DRAM") as dram,
            tc.tile_pool(name="pRT", bufs=5) as pRT,
            tc.tile_pool(name="pHL", bufs=4) as pHL,
            tc.tile_pool(name="pP", bufs=5) as pP,
            tc.tile_pool(name="pW", bufs=5) as pW,
            tc.tile_pool(name="pWT", bufs=5) as pWT,
            tc.tile_pool(name="small", bufs=10) as small,
            tc.tile_pool(name="psD", bufs=2, space="PSUM") as psD,
            tc.tile_pool(name="psT", bufs=2, space="PSUM") as psT,
            tc.tile_pool(name="psS", bufs=2, space="PSUM") as psS,
            tc.tile_pool(name="psK3", bufs=1, space="PSUM") as psK3,
        ):
            # ---------------- constants ----------------
            ident_bf = pers.tile([128, 128], bf16)
            make_identity(nc, ident_bf)
            ident_f32 = pers.tile([128, 128], f32)
            make_identity(nc, ident_f32)
            ones128 = pers.tile([128, 1], f32)
            nc.vector.memset(ones128[:], 1.0)

            # ---------------- persistent loads ----------------
            def load_qt(src, tag):
                t = pers.tile([16, QPC], bf16, tag=tag)
                nc.sync.dma_start(t[:], src[:])
                return t

            def load_rtc(src, tag):
                t = pers.tile([16, CORE_COLS], bf16, tag=tag)
                nc.sync.dma_start(t[:], src[:])
                return t

            QT1 = load_qt(qt1, "QT1")
            QT2 = load_qt(qt2, "QT2")
            QT3 = load_qt(qt3, "QT3")
            RTC = {1: load_rtc(rtc1, "RTC1"), 2: load_rtc(rtc2, "RTC2")}

            def load_hlc(src, tag):
                t = pers.tile([128, CORE_COLS // 128, 8], bf16, tag=tag)
                nc.sync.dma_start(
                    t[:], src.rearrange("(ci p) c -> p ci c", p=128))
                return t

            HLC = {1: load_hlc(hlc1, "HLC1"), 2: load_hlc(hlc2, "HLC2")}

            def load_128(src, ncol, dt, tag):
                t = pers.tile([128, ncol], dt, tag=tag)
                nc.sync.dma_start(t[:], src[:])
                return t

            Q2 = {1: load_128(q2_1, NQT, f32, "q21"),
                  2: load_128(q2_2, NQT, f32, "q22"),
                  3: load_128(q2_3, NQT, f32, "q23")}
            I3 = load_128(i3_d, NQT, mybir.dt.int16, "i3")

            def load_wide3(src, tag):
                t = pers.tile([128, NQT, 3], f32, tag=tag)
                nc.sync.dma_start(t[:], src.rearrange("(t p) c -> p t c", p=128))
                return t

            CTR1 = load_wide3(ctr1, "ctr1")
            CTR2 = load_wide3(ctr2, "ctr2")

            # negated q2 (evac bias) per stage: [128, NQT]
            def neg_of(t, tag):
                o = pers.tile([128, NQT], f32, tag=tag)
                nc.vector.tensor_scalar_mul(o[:], t[:], -1.0)
                return o

            NQ2 = {k: neg_of(v, f"nq2{k}") for k, v in Q2.items()}

            ag_in = dram.tile([128, NQT, 3], f32)
            ag_out = dram.tile([512, NQT, 3], f32)

            # ---------------- curv stage tile ----------------
            def emit_curv(t, qt_sb, stage, out_cb):
                W = W1
                CW = C1
                RT = RTC[stage]
                HL = HLC[stage]

                pd = psD.tile([128, W], f32, tag="pd")
                for g0 in range(0, W, 512):
                    gw = min(512, W - g0)
                    nc.tensor.matmul(
                        pd[:, g0:g0 + gw],
                        qt_sb[:, t * 128:(t + 1) * 128],
                        RT[:, t * 128 + g0:t * 128 + g0 + gw],
                        start=True, stop=True)
                P = pP.tile([128, W], bf16, tag="P")
                nc.scalar.activation(P[:], pd[:], Act.Identity, scale=2.0,
                                     bias=NQ2[stage][:, t:t + 1])
                v8 = small.tile([128, 8], f32, tag="v8")
                nc.vector.max(v8[:], P[:])
                P2 = pP.tile([128, W], bf16, tag="P2")
                nc.vector.match_replace(P2[:], v8[:], P[:], NEG_BIG)
                v16 = small.tile([128, 8], f32, tag="v16")
                nc.vector.max(v16[:], P2[:])
                thr = small.tile([128, 1], f32, tag="thr")
                nc.vector.tensor_scalar_max(thr[:], v16[:, 1:2], -RADIUS)
                Wm = pW.tile([128, W], bf16, tag="W")
                nc.gpsimd.tensor_scalar(Wm[:], P[:], thr[:], None, Alu.is_ge)
                ps = psS.tile([128, 4], f32, tag="psumS")
                pt = psT.tile([128, CW, 128], bf16, tag="psumT")
                for j in range(CW):
                    nc.tensor.transpose(pt[:, j, :],
                                        Wm[:, j * 128:(j + 1) * 128],
                                        ident_bf[:])
                WT = pWT.tile([128, CW, 128], bf16, tag="WT")
                nc.scalar.activation(WT[:], pt[:], Act.Copy)
                # hi and lo halves accumulate into the same 4 psum columns
                for j in range(CW):
                    for h in range(2):
                        nc.tensor.matmul(
                            ps[:], WT[:, j, :],
                            HL[:, t + j, h * 4:(h + 1) * 4],
                            start=(j == 0 and h == 0),
                            stop=(j == CW - 1 and h == 1))
                S4 = small.tile([128, 4], f32, tag="S4")
                nc.scalar.activation(S4[:], ps[:], Act.Copy, scale=-1.0 / 9.0)
                ctr = CTR1 if stage == 1 else CTR2
                # S4 = -[sum,count]/9, so ctr*S4[3]-S4[0:3] = (sum-count*ctr)/9
                curv = small.tile([128, 3], f32, tag="curv")
                nc.vector.scalar_tensor_tensor(
                    curv[:], ctr[:, t, :], S4[:, 3:4], S4[:, 0:3],
                    Alu.mult, Alu.subtract)
                out_cb(curv, v8, v16)

            # ---------------- KNN1: curvature of target ----------------
            for t in range(NQT):
                def cb1(curv, v8, v16, t=t):
                    nc.sync.dma_start(ag_in[:, t, :], curv[:])
                    if debug:
                        nc.sync.dma_start(
                            dbg["curv2_part"][t * 128:(t + 1) * 128, :],
                            curv[:])
                        nc.sync.dma_start(
                            dbg["v16_dbg"][t * 128:(t + 1) * 128, 0:8], v8[:])
                        nc.sync.dma_start(
                            dbg["v16_dbg"][t * 128:(t + 1) * 128, 8:16],
                            v16[:])
                emit_curv(t, QT1, 1, cb1)

            # ---------------- KNN2: curvature of source/warped ----------
            # (AllGather issued after tile 4 so it overlaps KNN2/KNN3 compute
            #  without stalling the gpsimd queue at a stage boundary.)
            for t in range(NQT):
                def cb2(curv, v8, v16, t=t):
                    nc.sync.dma_start(moved_out[t * 128:(t + 1) * 128, :],
                                      curv[:])
                emit_curv(t, QT2, 2, cb2)
                if t == 0:
                    nc.gpsimd.collective_compute(
                        "AllGather", mybir.AluOpType.bypass,
                        replica_groups=[[0, 1, 2, 3], [4, 5, 6, 7]],
                        ins=[ag_in.opt()], outs=[ag_out.opt()])


            # curv2 gather table [l, h, 4] = [curv2_bf16(3), 1]; bf16-only is
            # fine here (normalized weighted average, no cancellation) and
            # building it gpsimd-only avoids DVE head-of-line blocking while
            # the AllGather completes.
            Ctab_f = pers.tile([128, CH, 3], f32)
            nc.sync.dma_start(
                Ctab_f[:].rearrange("p (k t) c -> p k t c", k=4),
                ag_out[:].rearrange("(k p) t c -> p k t c", p=128))
            CtabHL = pers.tile([128, CH, 4], bf16)
            nc.vector.tensor_copy(CtabHL[:, :, 0:3], Ctab_f[:])
            nc.vector.memset(CtabHL[:, :, 3:4], 1.0)

            # ---------------- KNN3: interp k=5 ----------------
            for t in range(NQT):
                RT = pRT.tile([16, W3], bf16, tag="rt3")
                nc.sync.dma_start(RT[:], rt3[t])
                RTf = RT[:]
                P = pP.tile([128, W3], bf16, tag="P3")
                dd = pP.tile([128, W3], bf16, tag="dd")
                for h0 in range(0, W3, 1024):
                    hw = min(1024, W3 - h0)
                    pd = psD.tile([128, 1024], f32, tag="pd")
                    for g0 in range(0, hw, 512):
                        gw = min(512, hw - g0)
                        nc.tensor.matmul(
                            pd[:, g0:g0 + gw],
                            QT3[:, t * 128:(t + 1) * 128],
                            RTf[:, h0 + g0:h0 + g0 + gw],
                            start=True, stop=True)
                    nc.scalar.activation(
                        P[:, h0:h0 + hw], pd[:, 0:hw],
                        Act.Identity, scale=2.0, bias=NQ2[3][:, t:t + 1])
                    # dd = +sqdist directly from the same PSUM (ACT, not DVE)
                    nc.scalar.activation(
                        dd[:, h0:h0 + hw], pd[:, 0:hw],
                        Act.Identity, scale=-2.0, bias=Q2[3][:, t:t + 1])
                v8 = small.tile([128, 8], f32, tag="v8")
                nc.vector.max(v8[:], P[:])
                # window hl gather table [128, 16, 4] via ap_gather
                hlwin = pW.tile([128, 16, 4], bf16, tag="hlwin")
                nc.gpsimd.ap_gather(hlwin[:], CtabHL[:], I3[:, t:t + 1],
                                    128, CH, 4, 16)
                # weights from v8: alpha0' = sum of out-of-radius top-5 weights
                d8 = small.tile([128, 8], f32, tag="d8")
                nc.vector.tensor_scalar(d8[:], v8[:], -1.0, 1e-8, Alu.mult,
                                        Alu.add)
                w8 = small.tile([128, 5], f32, tag="w8")
                nc.vector.reciprocal(w8[:], d8[:, 0:5])
                w_out = small.tile([128, 5], f32, tag="w_out")
                nc.vector.scalar_tensor_tensor(w_out[:], d8[:, 0:5],
                                               RADIUS + 1e-8, w8[:],
                                               Alu.is_gt, Alu.mult)
                alpha0 = small.tile([128, 1], f32, tag="alpha0")
                nc.vector.tensor_reduce(alpha0[:], w_out[:], X, Alu.add)
                thrm = small.tile([128, 1], f32, tag="thrm")
                nc.vector.tensor_scalar_max(thrm[:], v8[:, 4:5], -RADIUS)
                # w_all = 1/dd (dd==0 impossible: cross-cloud, bf16 evac)
                w_all = pP.tile([128, W3], bf16, tag="w_all")
                with nc.allow_low_precision(reason="knn3 weights, 0.4% ok"):
                    nc.vector.reciprocal(w_all[:], dd[:])
                # U = w_all*1[P'>=thrm] + alpha0*1[P'>=v1]
                msk = pW.tile([128, W3], bf16, tag="msk")
                nc.gpsimd.tensor_scalar(msk[:], P[:], thrm[:], None, Alu.is_ge)
                U = pW.tile([128, W3], bf16, tag="U")
                nc.vector.tensor_mul(U[:], msk[:], w_all[:])
                M1 = pW.tile([128, W3], bf16, tag="M1")
                nc.vector.tensor_scalar(M1[:], P[:], v8[:, 0:1], alpha0[:],
                                        Alu.is_ge, Alu.mult)
                # transpose U and M1 chunks; both accumulate into one psum
                ps = psS.tile([128, 4], f32, tag="psumS")
                TCH = 2 * C3
                for half in range((TCH + 7) // 8):
                    nch = min(8, TCH - 8 * half)
                    pt = psT.tile([128, 8, 128], bf16, tag="psumT")
                    for j in range(nch):
                        c = half * 8 + j
                        src = U if c < C3 else M1
                        cc = c % C3
                        nc.tensor.transpose(
                            pt[:, j, :], src[:, cc * 128:(cc + 1) * 128],
                            ident_bf[:])
                    UT = pWT.tile([128, 8, 128], bf16, tag="WT")
                    nc.scalar.activation(UT[:, 0:nch, :], pt[:, 0:nch, :],
                                         Act.Copy)
                    for j in range(nch):
                        c = half * 8 + j
                        nc.tensor.matmul(ps[:], UT[:, j, :],
                                         hlwin[:, c % C3, :],
                                         start=(c == 0), stop=(c == TCH - 1))
                S4 = small.tile([128, 4], f32, tag="S4k3")
                nc.scalar.activation(S4[:], ps[:], Act.Copy)
                winv = small.tile([128, 1], f32, tag="winv")
                nc.vector.reciprocal(winv[:], S4[:, 3:4])
                inter = small.tile([128, 3], f32, tag="inter")
                nc.vector.tensor_scalar(inter[:], S4[:, 0:3], winv[:], None,
                                        Alu.mult)
                nc.sync.dma_start(inter_out[t * 128:(t + 1) * 128, :],
                                  inter[:])

    nc.compile()
    return nc


_CACHED = {}


def _get_program(debug=0):
    if debug not in _CACHED:
        _CACHED[debug] = _build(debug)
    return _CACHED[debug]


def kernel(pc_source, pc_target, pred_flow):
    from concourse.bass_utils import run_bass_kernel_spmd

    pc_source = np.asarray(pc_source, dtype=np.float32)
    pc_target = np.asarray(pc_target, dtype=np.float32)
    pred_flow = np.asarray(pred_flow, dtype=np.float32)
    nc = _get_program()
    in_maps, metas = prepare_inputs(pc_source, pc_target, pred_flow)
    res = run_bass_kernel_spmd(nc, in_maps, core_ids=list(range(NCORES)))
    total = 0.0
    for c in range(NCORES):
        moved = res.results[c]["moved_part"]
        inter = res.results[c]["inter_part"]
        diff = inter.astype(np.float64) - moved[metas[c]].astype(np.float64)
        total += float((diff ** 2).sum())
    return np.asarray(np.float32(total / B))
